# revision 21
# baseline (speedup 1.0000x reference)
"""Trainium2 Bass kernel for nn_NUFFTLayerMultiChannelInitMixed.

Math: the reference's spread->FFT->filter->IFFT->energy pipeline collapses to
an analytic-spectrum bilinear form. The Gaussian spread is deconvolved exactly
by the deconv^2 filter, so with ghat_n(k) ~ e^{-i k x_n} (alias images carry
weight e^{-tau(M-k)^2} ~ 3e-5 -- negligible vs the 2e-2 gate):

  e_i[n] = sum_k G_i(k) [cos(k x_n) C(k) + sin(k x_n) S(k)] + off_i
  C(k) = sum_n cos(k x_n),  S(k) = sum_n sin(k x_n)
  G_i = pref * w * deconv2 * mult_i * p^2  (~1/k^2 decay; K=64 keeps ~1.5e-4)

Layout: with K=64, cos rows and sin rows stack into ONE [128, N] matrix; the
+1/4-turn cos shift and a 3-way bf16 split of t (k*t_hi + k*t_mid + k*t_lo
exact in fp32 PSUM) ride in a single K=4 bf16 phase matmul per 512 cols.
Range-reduce (round-to-nearest via +MAGIC, alternating ACT/DVE to balance
engines), one Sin activation (bf16 out, accum_out = row sums for free), then
8 bf16 matmuls per batch (stationary = trig chunk, rhs = G*[C;S] [128,2])
yield energies directly in [n-part, channel] layout.
Sharding: batch-parallel, 2 of 16 batches per core, no collectives.
"""

import numpy as np

try:
    import concourse.bass as bass
except ImportError:
    import sys
    sys.path.insert(0, "/opt/trn_rl_repo")
    import concourse.bass as bass

import concourse.bacc as bacc
import concourse.mybir as mybir
from concourse import tile
from concourse.bass_utils import run_bass_kernel_spmd

F32 = mybir.dt.float32
BF16 = mybir.dt.bfloat16
AF = mybir.ActivationFunctionType
ALU = mybir.AluOpType

M = 2001
L = 2.0 * np.pi
TAU = 12.0 * (L / (2.0 * np.pi * M)) ** 2
K = 64                   # spectral truncation (1/k^2 filter decay)
N = 1024
B_FULL = 16
NCORES = 8
BPC = B_FULL // NCORES   # batches per core
NT = BPC * N             # 2048 points handled per core in one sweep
MAGIC = 12582912.0       # 1.5 * 2^23: (u + MAGIC) - MAGIC = round-to-nearest(u)
PI = float(np.pi)


def _bf16(a):
    a32 = np.asarray(a, dtype=np.float32)
    u32 = a32.view(np.uint32).astype(np.uint64)
    return (((u32 + 0x7FFF + ((u32 >> 16) & 1)) & 0xFFFF0000)
            .astype(np.uint32)).view(np.float32)


def _host_constants(shift0, shift1, amp0, amp1):
    """fp64 host-side k-space weights -> cst2 [128, 2] and channel offsets."""
    k = np.arange(K, dtype=np.float64)
    tau = float(TAU)
    p2 = np.exp(-2.0 * tau * k * k)
    deconv2 = (np.pi / tau) * np.exp(2.0 * tau * k * k)
    mult1 = float(amp0) * (4.0 * np.pi) / (k * k + (1.0 * float(shift0)) ** 2)
    mult2 = float(amp1) * (4.0 * np.pi) / (k * k + (0.5 * float(shift1)) ** 2)
    w = np.full(K, 2.0)
    w[0] = 1.0
    Cc = (M / L) * np.sqrt(4.0 * np.pi * tau)
    scale = 1.0 / ((2.0 * np.pi * M / L) * (2.0 * np.pi))
    pref = scale * Cc * Cc / M
    G1 = pref * w * deconv2 * mult1 * p2
    G2 = pref * w * deconv2 * mult2 * p2

    cst2 = np.zeros((128, 2), dtype=np.float64)
    cst2[0:K, 0] = G1
    cst2[K:2 * K, 0] = G1
    cst2[0:K, 1] = G2
    cst2[K:2 * K, 1] = G2

    # k=0 rows: cos row (0) is identically 1, and the sin k=0 row (64) is
    # made identically 1 too by giving it the +0.25 bias. Ride the constant
    # per-channel offset off_i = G_i[0]*N - sum(G_i) through them: row 0
    # carries the bf16-representable part, row 64 the residual, so no
    # precision is lost to UC's bf16 and no separate offset-add is needed.
    off1 = float(G1[0] * N - G1.sum())
    off2 = float(G2[0] * N - G2.sum())
    for i, off in enumerate((off1, off2)):
        hi = float(_bf16(np.float32(off)))
        cst2[0, i] = hi / N
        cst2[K, i] = (off - hi) / N
    return cst2.astype(np.float32)


def _pack_t(t_row):
    """[NT] fp32 t values -> [4, 128+NT] bf16: the phase-matmul stationary
    [k; k; k; bias] at cols 0:128 (so it lands first in the DMA), then the
    3-way split rows of t + ones row."""
    th = _bf16(t_row)
    tm = _bf16(t_row.astype(np.float64) - th.astype(np.float64))
    tl = _bf16(t_row.astype(np.float64) - th.astype(np.float64)
               - tm.astype(np.float64))
    ext = np.ones((4, 128 + NT), dtype=np.float32)
    ext[0, 128:] = th
    ext[1, 128:] = tm
    ext[2, 128:] = tl
    kv = np.concatenate([np.arange(K), np.arange(K)]).astype(np.float32)
    ext[0, :128] = kv
    ext[1, :128] = kv
    ext[2, :128] = kv
    bias = np.where(np.arange(128) < K, 0.25, 0.0)
    bias[K] = 0.25            # sin k=0 row -> constant 1, carries the offset
    ext[3, :128] = bias
    import ml_dtypes
    return ext.astype(ml_dtypes.bfloat16)


def _stv(tile_, start, step, num):
    """Strided [128, num] column view of a [128, *] tile."""
    ap = tile_[:]
    return bass.AP(ap.tensor, ap.offset + start, [ap.ap[0], [step, num]])


def _build_program(debug=False):
    nc = bacc.Bacc(None, target_bir_lowering=False, debug=debug)
    t_in = nc.declare_dram_parameter("t", [4, 128 + NT], BF16, isOutput=False)
    cst_in = nc.declare_dram_parameter("cst2", [128, 2], F32, isOutput=False)
    out_t = nc.declare_dram_parameter("out", [128, 16 * BPC], F32, isOutput=True)

    NQ = NT // 512  # 512-col quarters through the phase/trig pipeline

    with tile.TileContext(nc) as tc:
        import contextlib
        with contextlib.ExitStack() as ctx:
            pc = ctx.enter_context(tc.tile_pool(name="const", bufs=1))
            wp = ctx.enter_context(tc.tile_pool(name="work", bufs=NQ))
            sp = ctx.enter_context(tc.tile_pool(name="small", bufs=1))
            ps_u = ctx.enter_context(tc.tile_pool(name="psu", bufs=NQ, space="PSUM"))
            ps_T = ctx.enter_context(tc.tile_pool(name="psT", bufs=1, space="PSUM"))
            ps_d = ctx.enter_context(tc.tile_pool(name="psd", bufs=1, space="PSUM"))

            # Dummy Sin on scratch: makes the FIRST ScalarE op a Sin so the
            # compiler resident-set pick contains sin (its sets also contain
            # identity), avoiding a 1.3us mid-pipeline ACT_TABLE_LOAD swap.
            dummy = sp.tile([1, 2], F32, tag="dummy")
            nc.vector.memset(dummy[:], 0.0)
            dummy2 = sp.tile([1, 2], F32, tag="dummy2")
            nc.scalar.activation(dummy2[:], dummy[:], AF.Sin, scale=1.0)
            magicc = pc.tile([128, 1], F32, tag="magic")
            nc.gpsimd.memset(magicc[:], MAGIC)

            t_ext = pc.tile([4, 128 + NT], BF16, tag="t")
            nc.sync.dma_start(t_ext[:, 0:640], t_in[:, 0:640])
            nc.sync.dma_start(t_ext[:, 640:1664], t_in[:, 640:1664])
            nc.sync.dma_start(t_ext[:, 1664:], t_in[:, 1664:])
            cst2 = pc.tile([128, 2], F32, tag="cst2")
            nc.sync.dma_start(cst2[:], cst_in[:])
            kvb = t_ext[:, 0:128]

            # PE warm-up: ~2.1us of dummy matmuls during the DMA-in window
            # keep the HAM activity monitor busy so the real matmuls run at
            # 2.4 GHz (K=8/8) instead of the cold 1.2 GHz default.
            scr = sp.tile([128, 512], BF16, tag="scr")
            nc.vector.memset(scr[:], 1.0)
            dps = ps_d.tile([1, 512], F32, tag="dps")
            for _ in range(5):
                nc.tensor.matmul(dps[:], scr[:, 0:1], scr[:], start=True,
                                 stop=True)

            CS = sp.tile([128, NT], BF16, tag="CS")
            csum = sp.tile([128, NQ], F32, tag="csum")

            for q in range(NQ):
                sl = slice(512 * q, 512 * (q + 1))
                tsl = slice(128 + 512 * q, 128 + 512 * (q + 1))
                u = ps_u.tile([128, 512], F32, tag="u")
                nc.tensor.matmul(u[:], kvb, t_ext[:, tsl], start=True, stop=True)
                if q % 2 == 0:
                    # ACT-led reduction: rniM = u + MAGIC (rounded), on ScalarE
                    rniM = wp.tile([128, 512], F32, tag="rniM")
                    nc.scalar.activation(rniM[:], u[:], AF.Identity,
                                         bias=magicc[:])
                    negr = wp.tile([128, 512], F32, tag="negr")
                    nc.vector.scalar_tensor_tensor(negr[:], rniM[:], MAGIC, u[:],
                                                   ALU.subtract, ALU.subtract)
                    nc.scalar.activation(CS[:, sl], negr[:], AF.Sin,
                                         scale=-2.0 * PI,
                                         accum_out=csum[:, q:q + 1])
                else:
                    # DVE-led reduction
                    rni = wp.tile([128, 512], F32, tag="rni")
                    nc.vector.tensor_scalar(rni[:], u[:], MAGIC, MAGIC,
                                            ALU.add, ALU.subtract)
                    r = wp.tile([128, 512], F32, tag="r")
                    nc.vector.tensor_sub(r[:], u[:], rni[:])
                    nc.scalar.activation(CS[:, sl], r[:], AF.Sin,
                                         scale=2.0 * PI,
                                         accum_out=csum[:, q:q + 1])

            pT = ps_T.tile([128, 16 * BPC], F32, tag="pT")
            e = sp.tile([128, 16 * BPC], F32, tag="e")
            QB = NQ // BPC  # quarters per batch
            for b in range(BPC):
                # UC = cst2 * (csum_q0 + csum_q1), staged so only the stt is
                # on the critical path after the batch's last accum lands
                UCa = sp.tile([128, 2], F32, tag=f"UCa{b}")
                nc.gpsimd.tensor_scalar(UCa[:], cst2[:],
                                        csum[:, QB * b:QB * b + 1],
                                        None, ALU.mult)
                UC = sp.tile([128, 2], BF16, tag=f"UC{b}")
                nc.vector.scalar_tensor_tensor(
                    UC[:], cst2[:], csum[:, QB * b + 1:QB * b + 2], UCa[:],
                    ALU.mult, ALU.add)
                for j in range(8):
                    lh = CS[:, N * b + 128 * j: N * b + 128 * (j + 1)]
                    nc.tensor.matmul(pT[:, 16 * b + 2 * j: 16 * b + 2 * j + 2],
                                     lh, UC[:], start=True, stop=True)
                nc.vector.tensor_copy(e[:, 16 * b:16 * (b + 1)],
                                      pT[:, 16 * b:16 * (b + 1)])
                nc.sync.dma_start(out_t[:, 16 * b:16 * (b + 1)],
                                  e[:, 16 * b:16 * (b + 1)])
    return nc


def kernel(x, shift0, shift1, amp0, amp1):
    x = np.asarray(x, dtype=np.float32)
    cst2 = _host_constants(
        np.asarray(shift0).reshape(-1)[0], np.asarray(shift1).reshape(-1)[0],
        np.asarray(amp0).reshape(-1)[0], np.asarray(amp1).reshape(-1)[0])
    nc = _build_program()
    nc.finalize()

    t_full = (x.astype(np.float64) / (2.0 * np.pi)).astype(np.float32)
    in_maps = []
    for c in range(NCORES):
        t_ext = _pack_t(t_full[BPC * c: BPC * (c + 1)].reshape(NT))
        in_maps.append({"t": t_ext, "cst2": cst2})
    res = run_bass_kernel_spmd(nc, in_maps, list(range(NCORES)))
    outs = []
    for c in range(NCORES):
        arr = res.results[c]["out"]                      # [128, 16*BPC]
        arr = arr.reshape(128, BPC, 8, 2)                # (p, b, j, i)
        outs.append(arr.transpose(1, 2, 0, 3).reshape(BPC, N, 2))
    return np.concatenate(outs, axis=0).astype(np.float32)


# revision 23
# speedup vs baseline: 1.0348x; 1.0348x over previous
"""Trainium2 Bass kernel for nn_NUFFTLayerMultiChannelInitMixed.

Math: the reference's spread->FFT->filter->IFFT->energy pipeline collapses to
an analytic-spectrum bilinear form. The Gaussian spread is deconvolved exactly
by the deconv^2 filter, so with ghat_n(k) ~ e^{-i k x_n} (alias images carry
weight e^{-tau(M-k)^2} ~ 3e-5 -- negligible vs the 2e-2 gate):

  e_i[n] = sum_k G_i(k) [cos(k x_n) C(k) + sin(k x_n) S(k)] + off_i
  C(k) = sum_n cos(k x_n),  S(k) = sum_n sin(k x_n)
  G_i = pref * w * deconv2 * mult_i * p^2  (~1/k^2 decay; K=64 keeps ~1.5e-4)

Layout: with K=64, cos rows and sin rows stack into ONE [128, N] matrix; the
+1/4-turn cos shift and a 3-way bf16 split of t (k*t_hi + k*t_mid + k*t_lo
exact in fp32 PSUM) ride in a single K=4 bf16 phase matmul per 512 cols.
Range-reduce (round-to-nearest via +MAGIC, alternating ACT/DVE to balance
engines), one Sin activation (bf16 out, accum_out = row sums for free), then
8 bf16 matmuls per batch (stationary = trig chunk, rhs = G*[C;S] [128,2])
yield energies directly in [n-part, channel] layout.
Sharding: batch-parallel, 2 of 16 batches per core, no collectives.
"""

import numpy as np

try:
    import concourse.bass as bass
except ImportError:
    import sys
    sys.path.insert(0, "/opt/trn_rl_repo")
    import concourse.bass as bass

import concourse.bacc as bacc
import concourse.mybir as mybir
from concourse import tile
from concourse.bass_utils import run_bass_kernel_spmd

F32 = mybir.dt.float32
BF16 = mybir.dt.bfloat16
AF = mybir.ActivationFunctionType
ALU = mybir.AluOpType

M = 2001
L = 2.0 * np.pi
TAU = 12.0 * (L / (2.0 * np.pi * M)) ** 2
K = 64                   # spectral truncation (1/k^2 filter decay)
N = 1024
B_FULL = 16
NCORES = 8
BPC = B_FULL // NCORES   # batches per core
NT = BPC * N             # 2048 points handled per core in one sweep
MAGIC = 12582912.0       # 1.5 * 2^23: (u + MAGIC) - MAGIC = round-to-nearest(u)
PI = float(np.pi)


def _bf16(a):
    a32 = np.asarray(a, dtype=np.float32)
    u32 = a32.view(np.uint32).astype(np.uint64)
    return (((u32 + 0x7FFF + ((u32 >> 16) & 1)) & 0xFFFF0000)
            .astype(np.uint32)).view(np.float32)


def _host_constants(shift0, shift1, amp0, amp1):
    """fp64 host-side k-space weights -> cst2 [128, 2] and channel offsets."""
    k = np.arange(K, dtype=np.float64)
    tau = float(TAU)
    p2 = np.exp(-2.0 * tau * k * k)
    deconv2 = (np.pi / tau) * np.exp(2.0 * tau * k * k)
    mult1 = float(amp0) * (4.0 * np.pi) / (k * k + (1.0 * float(shift0)) ** 2)
    mult2 = float(amp1) * (4.0 * np.pi) / (k * k + (0.5 * float(shift1)) ** 2)
    w = np.full(K, 2.0)
    w[0] = 1.0
    Cc = (M / L) * np.sqrt(4.0 * np.pi * tau)
    scale = 1.0 / ((2.0 * np.pi * M / L) * (2.0 * np.pi))
    pref = scale * Cc * Cc / M
    G1 = pref * w * deconv2 * mult1 * p2
    G2 = pref * w * deconv2 * mult2 * p2

    cst2 = np.zeros((128, 2), dtype=np.float64)
    cst2[0:K, 0] = G1
    cst2[K:2 * K, 0] = G1
    cst2[0:K, 1] = G2
    cst2[K:2 * K, 1] = G2

    # k=0 rows: cos row (0) is identically 1, and the sin k=0 row (64) is
    # made identically 1 too by giving it the +0.25 bias. Ride the constant
    # per-channel offset off_i = G_i[0]*N - sum(G_i) through them: row 0
    # carries the bf16-representable part, row 64 the residual, so no
    # precision is lost to UC's bf16 and no separate offset-add is needed.
    off1 = float(G1[0] * N - G1.sum())
    off2 = float(G2[0] * N - G2.sum())
    for i, off in enumerate((off1, off2)):
        hi = float(_bf16(np.float32(off)))
        cst2[0, i] = hi / N
        cst2[K, i] = (off - hi) / N
    return cst2.astype(np.float32)


def _pack_t(t_row):
    """[NT] fp32 t values -> [4, 128+NT] bf16: the phase-matmul stationary
    [k; k; k; bias] at cols 0:128 (so it lands first in the DMA), then the
    3-way split rows of t + ones row."""
    th = _bf16(t_row)
    tm = _bf16(t_row.astype(np.float64) - th.astype(np.float64))
    tl = _bf16(t_row.astype(np.float64) - th.astype(np.float64)
               - tm.astype(np.float64))
    ext = np.ones((4, 128 + NT), dtype=np.float32)
    ext[0, 128:] = th
    ext[1, 128:] = tm
    ext[2, 128:] = tl
    kv = np.concatenate([np.arange(K), np.arange(K)]).astype(np.float32)
    ext[0, :128] = kv
    ext[1, :128] = kv
    ext[2, :128] = kv
    bias = np.where(np.arange(128) < K, 0.25, 0.0)
    bias[K] = 0.25            # sin k=0 row -> constant 1, carries the offset
    ext[3, :128] = bias
    import ml_dtypes
    return ext.astype(ml_dtypes.bfloat16)


def _stv(tile_, start, step, num):
    """Strided [128, num] column view of a [128, *] tile."""
    ap = tile_[:]
    return bass.AP(ap.tensor, ap.offset + start, [ap.ap[0], [step, num]])


def _build_program(debug=False):
    nc = bacc.Bacc(None, target_bir_lowering=False, debug=debug)
    t_in = nc.declare_dram_parameter("t", [4, 128 + NT], BF16, isOutput=False)
    cst_in = nc.declare_dram_parameter("cst2", [128, 2], F32, isOutput=False)
    out_t = nc.declare_dram_parameter("out", [128, 16 * BPC], F32, isOutput=True)

    NQ = NT // 512  # 512-col quarters through the phase/trig pipeline

    with tile.TileContext(nc) as tc:
        import contextlib
        with contextlib.ExitStack() as ctx:
            pc = ctx.enter_context(tc.tile_pool(name="const", bufs=1))
            wp = ctx.enter_context(tc.tile_pool(name="work", bufs=NQ))
            sp = ctx.enter_context(tc.tile_pool(name="small", bufs=1))
            ps_u = ctx.enter_context(tc.tile_pool(name="psu", bufs=NQ, space="PSUM"))
            ps_T = ctx.enter_context(tc.tile_pool(name="psT", bufs=1, space="PSUM"))

            # Dummy Sin on scratch: makes the FIRST ScalarE op a Sin so the
            # compiler resident-set pick contains sin (its sets also contain
            # identity), avoiding a 1.3us mid-pipeline ACT_TABLE_LOAD swap.
            dummy = sp.tile([1, 2], F32, tag="dummy")
            nc.vector.memset(dummy[:], 0.0)
            dummy2 = sp.tile([1, 2], F32, tag="dummy2")
            nc.scalar.activation(dummy2[:], dummy[:], AF.Sin, scale=1.0)
            magicc = pc.tile([128, 1], F32, tag="magic")
            nc.gpsimd.memset(magicc[:], MAGIC)

            t_ext = pc.tile([4, 128 + NT], BF16, tag="t")
            nc.sync.dma_start(t_ext[:, 0:640], t_in[:, 0:640])
            nc.sync.dma_start(t_ext[:, 640:1664], t_in[:, 640:1664])
            nc.sync.dma_start(t_ext[:, 1664:], t_in[:, 1664:])
            cst2 = pc.tile([128, 2], F32, tag="cst2")
            nc.sync.dma_start(cst2[:], cst_in[:])
            kvb = t_ext[:, 0:128]

            CS = sp.tile([128, NT], BF16, tag="CS")
            csum = sp.tile([128, NQ], F32, tag="csum")

            for q in range(NQ):
                sl = slice(512 * q, 512 * (q + 1))
                tsl = slice(128 + 512 * q, 128 + 512 * (q + 1))
                u = ps_u.tile([128, 512], F32, tag="u")
                nc.tensor.matmul(u[:], kvb, t_ext[:, tsl], start=True, stop=True)
                if q % 2 == 0:
                    # ACT-led reduction: rniM = u + MAGIC (rounded), on ScalarE
                    rniM = wp.tile([128, 512], F32, tag="rniM")
                    nc.scalar.activation(rniM[:], u[:], AF.Identity,
                                         bias=magicc[:])
                    negr = wp.tile([128, 512], F32, tag="negr")
                    nc.vector.scalar_tensor_tensor(negr[:], rniM[:], MAGIC, u[:],
                                                   ALU.subtract, ALU.subtract)
                    nc.scalar.activation(CS[:, sl], negr[:], AF.Sin,
                                         scale=-2.0 * PI,
                                         accum_out=csum[:, q:q + 1])
                else:
                    # DVE-led reduction
                    rni = wp.tile([128, 512], F32, tag="rni")
                    nc.vector.tensor_scalar(rni[:], u[:], MAGIC, MAGIC,
                                            ALU.add, ALU.subtract)
                    r = wp.tile([128, 512], F32, tag="r")
                    nc.vector.tensor_sub(r[:], u[:], rni[:])
                    nc.scalar.activation(CS[:, sl], r[:], AF.Sin,
                                         scale=2.0 * PI,
                                         accum_out=csum[:, q:q + 1])

            pT = ps_T.tile([128, 16 * BPC], F32, tag="pT")
            e = sp.tile([128, 16 * BPC], F32, tag="e")
            QB = NQ // BPC  # quarters per batch
            for b in range(BPC):
                # UC = cst2 * (csum_q0 + csum_q1), staged so only the stt is
                # on the critical path after the batch's last accum lands
                UCa = sp.tile([128, 2], F32, tag=f"UCa{b}")
                nc.gpsimd.tensor_scalar(UCa[:], cst2[:],
                                        csum[:, QB * b:QB * b + 1],
                                        None, ALU.mult)
                UC = sp.tile([128, 2], BF16, tag=f"UC{b}")
                nc.vector.scalar_tensor_tensor(
                    UC[:], cst2[:], csum[:, QB * b + 1:QB * b + 2], UCa[:],
                    ALU.mult, ALU.add)
                for j in range(8):
                    lh = CS[:, N * b + 128 * j: N * b + 128 * (j + 1)]
                    nc.tensor.matmul(pT[:, 16 * b + 2 * j: 16 * b + 2 * j + 2],
                                     lh, UC[:], start=True, stop=True)
                nc.vector.tensor_copy(e[:, 16 * b:16 * (b + 1)],
                                      pT[:, 16 * b:16 * (b + 1)])
                nc.sync.dma_start(out_t[:, 16 * b:16 * (b + 1)],
                                  e[:, 16 * b:16 * (b + 1)])
    return nc


def kernel(x, shift0, shift1, amp0, amp1):
    x = np.asarray(x, dtype=np.float32)
    cst2 = _host_constants(
        np.asarray(shift0).reshape(-1)[0], np.asarray(shift1).reshape(-1)[0],
        np.asarray(amp0).reshape(-1)[0], np.asarray(amp1).reshape(-1)[0])
    nc = _build_program()
    nc.finalize()

    t_full = (x.astype(np.float64) / (2.0 * np.pi)).astype(np.float32)
    in_maps = []
    for c in range(NCORES):
        t_ext = _pack_t(t_full[BPC * c: BPC * (c + 1)].reshape(NT))
        in_maps.append({"t": t_ext, "cst2": cst2})
    res = run_bass_kernel_spmd(nc, in_maps, list(range(NCORES)))
    outs = []
    for c in range(NCORES):
        arr = res.results[c]["out"]                      # [128, 16*BPC]
        arr = arr.reshape(128, BPC, 8, 2)                # (p, b, j, i)
        outs.append(arr.transpose(1, 2, 0, 3).reshape(BPC, N, 2))
    return np.concatenate(outs, axis=0).astype(np.float32)


# revision 24
# speedup vs baseline: 1.0437x; 1.0086x over previous
"""Trainium2 Bass kernel for nn_NUFFTLayerMultiChannelInitMixed.

Math: the reference's spread->FFT->filter->IFFT->energy pipeline collapses to
an analytic-spectrum bilinear form. The Gaussian spread is deconvolved exactly
by the deconv^2 filter, so with ghat_n(k) ~ e^{-i k x_n} (alias images carry
weight e^{-tau(M-k)^2} ~ 3e-5 -- negligible vs the 2e-2 gate):

  e_i[n] = sum_k G_i(k) [cos(k x_n) C(k) + sin(k x_n) S(k)] + off_i
  C(k) = sum_n cos(k x_n),  S(k) = sum_n sin(k x_n)
  G_i = pref * w * deconv2 * mult_i * p^2  (~1/k^2 decay; K=64 keeps ~1.5e-4)

Layout: with K=64, cos rows and sin rows stack into ONE [128, N] matrix; the
+1/4-turn cos shift and a 3-way bf16 split of t (k*t_hi + k*t_mid + k*t_lo
exact in fp32 PSUM) ride in a single K=4 bf16 phase matmul per 512 cols.
Range-reduce (round-to-nearest via +MAGIC, alternating ACT/DVE to balance
engines), one Sin activation (bf16 out, accum_out = row sums for free), then
8 bf16 matmuls per batch (stationary = trig chunk, rhs = G*[C;S] [128,2])
yield energies directly in [n-part, channel] layout.
Sharding: batch-parallel, 2 of 16 batches per core, no collectives.
"""

import numpy as np

try:
    import concourse.bass as bass
except ImportError:
    import sys
    sys.path.insert(0, "/opt/trn_rl_repo")
    import concourse.bass as bass

import concourse.bacc as bacc
import concourse.mybir as mybir
from concourse import tile
from concourse.bass_utils import run_bass_kernel_spmd

F32 = mybir.dt.float32
BF16 = mybir.dt.bfloat16
AF = mybir.ActivationFunctionType
ALU = mybir.AluOpType

M = 2001
L = 2.0 * np.pi
TAU = 12.0 * (L / (2.0 * np.pi * M)) ** 2
K = 64                   # spectral truncation (1/k^2 filter decay)
N = 1024
B_FULL = 16
NCORES = 8
BPC = B_FULL // NCORES   # batches per core
NT = BPC * N             # 2048 points handled per core in one sweep
MAGIC = 12582912.0       # 1.5 * 2^23: (u + MAGIC) - MAGIC = round-to-nearest(u)
PI = float(np.pi)


def _bf16(a):
    a32 = np.asarray(a, dtype=np.float32)
    u32 = a32.view(np.uint32).astype(np.uint64)
    return (((u32 + 0x7FFF + ((u32 >> 16) & 1)) & 0xFFFF0000)
            .astype(np.uint32)).view(np.float32)


def _host_constants(shift0, shift1, amp0, amp1):
    """fp64 host-side k-space weights -> cst2 [128, 2] and channel offsets."""
    k = np.arange(K, dtype=np.float64)
    tau = float(TAU)
    p2 = np.exp(-2.0 * tau * k * k)
    deconv2 = (np.pi / tau) * np.exp(2.0 * tau * k * k)
    mult1 = float(amp0) * (4.0 * np.pi) / (k * k + (1.0 * float(shift0)) ** 2)
    mult2 = float(amp1) * (4.0 * np.pi) / (k * k + (0.5 * float(shift1)) ** 2)
    w = np.full(K, 2.0)
    w[0] = 1.0
    Cc = (M / L) * np.sqrt(4.0 * np.pi * tau)
    scale = 1.0 / ((2.0 * np.pi * M / L) * (2.0 * np.pi))
    pref = scale * Cc * Cc / M
    G1 = pref * w * deconv2 * mult1 * p2
    G2 = pref * w * deconv2 * mult2 * p2

    cst2 = np.zeros((128, 2), dtype=np.float64)
    cst2[0:K, 0] = G1
    cst2[K:2 * K, 0] = G1
    cst2[0:K, 1] = G2
    cst2[K:2 * K, 1] = G2

    # k=0 rows: cos row (0) is identically 1, and the sin k=0 row (64) is
    # made identically 1 too by giving it the +0.25 bias. Ride the constant
    # per-channel offset off_i = G_i[0]*N - sum(G_i) through them: row 0
    # carries the bf16-representable part, row 64 the residual, so no
    # precision is lost to UC's bf16 and no separate offset-add is needed.
    off1 = float(G1[0] * N - G1.sum())
    off2 = float(G2[0] * N - G2.sum())
    for i, off in enumerate((off1, off2)):
        hi = float(_bf16(np.float32(off)))
        cst2[0, i] = hi / N
        cst2[K, i] = (off - hi) / N
    return cst2.astype(np.float32)


def _pack_t(t_row):
    """[NT] fp32 t values -> [4, 128+NT] bf16: the phase-matmul stationary
    [k; k; k; bias] at cols 0:128 (so it lands first in the DMA), then the
    3-way split rows of t + ones row."""
    th = _bf16(t_row)
    tm = _bf16(t_row.astype(np.float64) - th.astype(np.float64))
    tl = _bf16(t_row.astype(np.float64) - th.astype(np.float64)
               - tm.astype(np.float64))
    ext = np.ones((4, 128 + NT), dtype=np.float32)
    ext[0, 128:] = th
    ext[1, 128:] = tm
    ext[2, 128:] = tl
    kv = np.concatenate([np.arange(K), np.arange(K)]).astype(np.float32)
    ext[0, :128] = kv
    ext[1, :128] = kv
    ext[2, :128] = kv
    bias = np.where(np.arange(128) < K, 0.25, 0.0)
    bias[K] = 0.25            # sin k=0 row -> constant 1, carries the offset
    ext[3, :128] = bias
    import ml_dtypes
    return ext.astype(ml_dtypes.bfloat16)


def _stv(tile_, start, step, num):
    """Strided [128, num] column view of a [128, *] tile."""
    ap = tile_[:]
    return bass.AP(ap.tensor, ap.offset + start, [ap.ap[0], [step, num]])


def _build_program(debug=False):
    nc = bacc.Bacc(None, target_bir_lowering=False, debug=debug)
    t_in = nc.declare_dram_parameter("t", [4, 128 + NT], BF16, isOutput=False)
    cst_in = nc.declare_dram_parameter("cst2", [128, 2], F32, isOutput=False)
    out_t = nc.declare_dram_parameter("out", [128, 16 * BPC], F32, isOutput=True)

    NQ = NT // 512  # 512-col quarters through the phase/trig pipeline

    with tile.TileContext(nc) as tc:
        import contextlib
        with contextlib.ExitStack() as ctx:
            pc = ctx.enter_context(tc.tile_pool(name="const", bufs=1))
            wp = ctx.enter_context(tc.tile_pool(name="work", bufs=NQ))
            sp = ctx.enter_context(tc.tile_pool(name="small", bufs=1))
            ps_u = ctx.enter_context(tc.tile_pool(name="psu", bufs=NQ, space="PSUM"))
            ps_T = ctx.enter_context(tc.tile_pool(name="psT", bufs=1, space="PSUM"))

            # Dummy Sin on scratch: makes the FIRST ScalarE op a Sin so the
            # compiler resident-set pick contains sin (its sets also contain
            # identity), avoiding a 1.3us mid-pipeline ACT_TABLE_LOAD swap.
            dummy = sp.tile([1, 2], F32, tag="dummy")
            nc.vector.memset(dummy[:], 0.0)
            dummy2 = sp.tile([1, 2], F32, tag="dummy2")
            nc.scalar.activation(dummy2[:], dummy[:], AF.Sin, scale=1.0)
            magicc = pc.tile([128, 1], F32, tag="magic")
            nc.gpsimd.memset(magicc[:], MAGIC)

            t_ext = pc.tile([4, 128 + NT], BF16, tag="t")
            nc.sync.dma_start(t_ext[:, 0:1152], t_in[:, 0:1152])
            nc.sync.dma_start(t_ext[:, 1152:], t_in[:, 1152:])
            cst2 = pc.tile([128, 2], F32, tag="cst2")
            nc.sync.dma_start(cst2[:], cst_in[:])
            kvb = t_ext[:, 0:128]

            CS = sp.tile([128, NT], BF16, tag="CS")
            csum = sp.tile([128, NQ], F32, tag="csum")

            for q in range(NQ):
                sl = slice(512 * q, 512 * (q + 1))
                tsl = slice(128 + 512 * q, 128 + 512 * (q + 1))
                u = ps_u.tile([128, 512], F32, tag="u")
                nc.tensor.matmul(u[:], kvb, t_ext[:, tsl], start=True, stop=True)
                if q % 2 == 0:
                    # ACT-led reduction: rniM = u + MAGIC (rounded), on ScalarE
                    rniM = wp.tile([128, 512], F32, tag="rniM")
                    nc.scalar.activation(rniM[:], u[:], AF.Identity,
                                         bias=magicc[:])
                    negr = wp.tile([128, 512], F32, tag="negr")
                    nc.vector.scalar_tensor_tensor(negr[:], rniM[:], MAGIC, u[:],
                                                   ALU.subtract, ALU.subtract)
                    nc.scalar.activation(CS[:, sl], negr[:], AF.Sin,
                                         scale=-2.0 * PI,
                                         accum_out=csum[:, q:q + 1])
                else:
                    # DVE-led reduction
                    rni = wp.tile([128, 512], F32, tag="rni")
                    nc.vector.tensor_scalar(rni[:], u[:], MAGIC, MAGIC,
                                            ALU.add, ALU.subtract)
                    r = wp.tile([128, 512], F32, tag="r")
                    nc.vector.tensor_sub(r[:], u[:], rni[:])
                    nc.scalar.activation(CS[:, sl], r[:], AF.Sin,
                                         scale=2.0 * PI,
                                         accum_out=csum[:, q:q + 1])

            pT = ps_T.tile([128, 16 * BPC], F32, tag="pT")
            e = sp.tile([128, 16 * BPC], F32, tag="e")
            QB = NQ // BPC  # quarters per batch
            for b in range(BPC):
                # UC = cst2 * (csum_q0 + csum_q1), staged so only the stt is
                # on the critical path after the batch's last accum lands
                UCa = sp.tile([128, 2], F32, tag=f"UCa{b}")
                nc.gpsimd.tensor_scalar(UCa[:], cst2[:],
                                        csum[:, QB * b:QB * b + 1],
                                        None, ALU.mult)
                UC = sp.tile([128, 2], BF16, tag=f"UC{b}")
                nc.vector.scalar_tensor_tensor(
                    UC[:], cst2[:], csum[:, QB * b + 1:QB * b + 2], UCa[:],
                    ALU.mult, ALU.add)
                for j in range(8):
                    lh = CS[:, N * b + 128 * j: N * b + 128 * (j + 1)]
                    nc.tensor.matmul(pT[:, 16 * b + 2 * j: 16 * b + 2 * j + 2],
                                     lh, UC[:], start=True, stop=True)
                nc.vector.tensor_copy(e[:, 16 * b:16 * (b + 1)],
                                      pT[:, 16 * b:16 * (b + 1)])
                nc.sync.dma_start(out_t[:, 16 * b:16 * (b + 1)],
                                  e[:, 16 * b:16 * (b + 1)])
    return nc


def kernel(x, shift0, shift1, amp0, amp1):
    x = np.asarray(x, dtype=np.float32)
    cst2 = _host_constants(
        np.asarray(shift0).reshape(-1)[0], np.asarray(shift1).reshape(-1)[0],
        np.asarray(amp0).reshape(-1)[0], np.asarray(amp1).reshape(-1)[0])
    nc = _build_program()
    nc.finalize()

    t_full = (x.astype(np.float64) / (2.0 * np.pi)).astype(np.float32)
    in_maps = []
    for c in range(NCORES):
        t_ext = _pack_t(t_full[BPC * c: BPC * (c + 1)].reshape(NT))
        in_maps.append({"t": t_ext, "cst2": cst2})
    res = run_bass_kernel_spmd(nc, in_maps, list(range(NCORES)))
    outs = []
    for c in range(NCORES):
        arr = res.results[c]["out"]                      # [128, 16*BPC]
        arr = arr.reshape(128, BPC, 8, 2)                # (p, b, j, i)
        outs.append(arr.transpose(1, 2, 0, 3).reshape(BPC, N, 2))
    return np.concatenate(outs, axis=0).astype(np.float32)


# revision 25
# speedup vs baseline: 1.1060x; 1.0597x over previous
"""Trainium2 Bass kernel for nn_NUFFTLayerMultiChannelInitMixed.

Math: the reference's spread->FFT->filter->IFFT->energy pipeline collapses to
an analytic-spectrum bilinear form. The Gaussian spread is deconvolved exactly
by the deconv^2 filter, so with ghat_n(k) ~ e^{-i k x_n} (alias images carry
weight e^{-tau(M-k)^2} ~ 3e-5 -- negligible vs the 2e-2 gate):

  e_i[n] = sum_k G_i(k) [cos(k x_n) C(k) + sin(k x_n) S(k)] + off_i
  C(k) = sum_n cos(k x_n),  S(k) = sum_n sin(k x_n)
  G_i = pref * w * deconv2 * mult_i * p^2  (~1/k^2 decay; K=32 keeps ~2e-4)

Layout: with K=32, BOTH batches pack into one [128, 1024] trig matrix --
row r: batch r//64, kind (r%64)//32 (cos/sin), k = r%32. One K=7 bf16 phase
matmul per 512 cols builds k*t (3-way bf16 split of t, exact in fp32 PSUM)
with the +1/4-turn cos bias riding in the contraction. Range-reduce
(round-to-nearest via +MAGIC, ACT-led and DVE-led halves to balance engines),
one Sin activation per half (bf16 out, accum_out = row sums for free), then
16 matmuls (stationary = 64-row trig chunk, rhs = G*[C;S] [64, 2]) yield
energies directly in [n-part, channel] layout. The constant per-channel
offset rides the two identically-1 trig rows (cos k=0, and sin k=0 via bias).
Sharding: batch-parallel, 2 of 16 batches per core, no collectives.
"""

import numpy as np

try:
    import concourse.bass as bass
except ImportError:
    import sys
    sys.path.insert(0, "/opt/trn_rl_repo")
    import concourse.bass as bass

import concourse.bacc as bacc
import concourse.mybir as mybir
from concourse import tile
from concourse.bass_utils import run_bass_kernel_spmd

F32 = mybir.dt.float32
BF16 = mybir.dt.bfloat16
AF = mybir.ActivationFunctionType
ALU = mybir.AluOpType

M = 2001
L = 2.0 * np.pi
TAU = 12.0 * (L / (2.0 * np.pi * M)) ** 2
K = 32                   # spectral truncation (1/k^2 filter decay)
N = 1024
B_FULL = 16
NCORES = 8
BPC = B_FULL // NCORES   # batches per core, packed into row halves
MAGIC = 12582912.0       # 1.5 * 2^23: (u + MAGIC) - MAGIC = round-to-nearest(u)
PI = float(np.pi)

_RB = np.arange(128) % 64            # within-batch row index
_KROW = _RB % K                      # k value per row
_BIAS = np.where(_RB <= K, 0.25, 0.0)  # cos rows + the sin k=0 offset row


def _bf16(a):
    a32 = np.asarray(a, dtype=np.float32)
    u32 = a32.view(np.uint32).astype(np.uint64)
    return (((u32 + 0x7FFF + ((u32 >> 16) & 1)) & 0xFFFF0000)
            .astype(np.uint32)).view(np.float32)


def _host_constants(shift0, shift1, amp0, amp1):
    """fp64 host-side k-space weights -> cst2 [128, 2]."""
    k = np.arange(K, dtype=np.float64)
    tau = float(TAU)
    p2 = np.exp(-2.0 * tau * k * k)
    deconv2 = (np.pi / tau) * np.exp(2.0 * tau * k * k)
    mult1 = float(amp0) * (4.0 * np.pi) / (k * k + (1.0 * float(shift0)) ** 2)
    mult2 = float(amp1) * (4.0 * np.pi) / (k * k + (0.5 * float(shift1)) ** 2)
    w = np.full(K, 2.0)
    w[0] = 1.0
    Cc = (M / L) * np.sqrt(4.0 * np.pi * tau)
    scale = 1.0 / ((2.0 * np.pi * M / L) * (2.0 * np.pi))
    pref = scale * Cc * Cc / M
    G1 = pref * w * deconv2 * mult1 * p2
    G2 = pref * w * deconv2 * mult2 * p2

    cst2 = np.zeros((128, 2), dtype=np.float64)
    cst2[:, 0] = G1[_KROW]
    cst2[:, 1] = G2[_KROW]

    # Constant offset off_i = G_i[0]*N - sum(G_i) rides the two rows that are
    # identically 1: cos k=0 (rb=0, bf16-representable part) and sin k=0
    # (rb=K, made 1 by its +0.25 bias; carries the residual) -- no separate
    # offset-add instruction and no bf16 precision loss.
    off1 = float(G1[0] * N - G1.sum())
    off2 = float(G2[0] * N - G2.sum())
    for i, off in enumerate((off1, off2)):
        hi = float(_bf16(np.float32(off)))
        cst2[_RB == 0, i] = hi / N
        cst2[_RB == K, i] = (off - hi) / N
    return cst2.astype(np.float32)


def _pack_t(t_rows):
    """[BPC, N] fp32 t values -> [7, 128+N] bf16: the phase stationary
    [7, 128] at cols 0:128 (first in the DMA), then per-batch 3-way split
    rows of t (rows 3b..3b+2) + ones row 6."""
    ext = np.ones((7, 128 + N), dtype=np.float32)
    for b in range(BPC):
        t = t_rows[b]
        th = _bf16(t)
        tm = _bf16(t.astype(np.float64) - th.astype(np.float64))
        tl = _bf16(t.astype(np.float64) - th.astype(np.float64)
                   - tm.astype(np.float64))
        ext[3 * b + 0, 128:] = th
        ext[3 * b + 1, 128:] = tm
        ext[3 * b + 2, 128:] = tl
    kvb = np.zeros((7, 128), dtype=np.float64)
    for b in range(BPC):
        rows = (np.arange(128) // 64) == b
        for j in range(3):
            kvb[3 * b + j, rows] = _KROW[rows]
    kvb[6] = _BIAS
    ext[:, :128] = kvb
    import ml_dtypes
    return ext.astype(ml_dtypes.bfloat16)


def _build_program(debug=False):
    nc = bacc.Bacc(None, target_bir_lowering=False, debug=debug)
    t_in = nc.declare_dram_parameter("t", [7, 128 + N], BF16, isOutput=False)
    cst_in = nc.declare_dram_parameter("cst2", [128, 2], F32, isOutput=False)
    out_t = nc.declare_dram_parameter("out", [128, 16 * BPC], F32, isOutput=True)

    with tile.TileContext(nc) as tc:
        import contextlib
        with contextlib.ExitStack() as ctx:
            pc = ctx.enter_context(tc.tile_pool(name="const", bufs=1))
            wp = ctx.enter_context(tc.tile_pool(name="work", bufs=2))
            sp = ctx.enter_context(tc.tile_pool(name="small", bufs=1))
            ps_u = ctx.enter_context(tc.tile_pool(name="psu", bufs=2, space="PSUM"))
            ps_T = ctx.enter_context(tc.tile_pool(name="psT", bufs=1, space="PSUM"))

            # Dummy Sin on scratch: makes the FIRST ScalarE op a Sin so the
            # compiler resident-set pick contains sin (its sets also contain
            # identity), avoiding a 1.3us mid-pipeline ACT_TABLE_LOAD swap.
            dummy = sp.tile([1, 2], F32, tag="dummy")
            nc.vector.memset(dummy[:], 0.0)
            dummy2 = sp.tile([1, 2], F32, tag="dummy2")
            nc.scalar.activation(dummy2[:], dummy[:], AF.Sin, scale=1.0)
            magicc = pc.tile([128, 1], F32, tag="magic")
            nc.gpsimd.memset(magicc[:], MAGIC)

            t_ext = pc.tile([7, 128 + N], BF16, tag="t")
            nc.sync.dma_start(t_ext[:, 0:640], t_in[:, 0:640])
            nc.sync.dma_start(t_ext[:, 640:], t_in[:, 640:])
            cst2 = pc.tile([128, 2], F32, tag="cst2")
            nc.sync.dma_start(cst2[:], cst_in[:])
            kvb = t_ext[:, 0:128]

            CS = sp.tile([128, N], BF16, tag="CS")
            csum = sp.tile([128, 2], F32, tag="csum")

            u0 = ps_u.tile([128, 512], F32, tag="u")
            nc.tensor.matmul(u0[:], kvb, t_ext[:, 128:640], start=True, stop=True)
            u1 = ps_u.tile([128, 512], F32, tag="u")
            nc.tensor.matmul(u1[:], kvb, t_ext[:, 640:1152], start=True, stop=True)

            # half 0: ACT-led range reduction (Identity reads PSUM, adds MAGIC)
            rniM = wp.tile([128, 512], F32, tag="rniM")
            nc.scalar.activation(rniM[:], u0[:], AF.Identity, bias=magicc[:])
            negr = wp.tile([128, 512], F32, tag="negr")
            nc.vector.scalar_tensor_tensor(negr[:], rniM[:], MAGIC, u0[:],
                                           ALU.subtract, ALU.subtract)
            nc.scalar.activation(CS[:, 0:512], negr[:], AF.Sin, scale=-2.0 * PI,
                                 accum_out=csum[:, 0:1])
            # half 1: DVE-led
            rni = wp.tile([128, 512], F32, tag="rni")
            nc.vector.tensor_scalar(rni[:], u1[:], MAGIC, MAGIC,
                                    ALU.add, ALU.subtract)
            r = wp.tile([128, 512], F32, tag="r")
            nc.vector.tensor_sub(r[:], u1[:], rni[:])
            nc.scalar.activation(CS[:, 512:1024], r[:], AF.Sin, scale=2.0 * PI,
                                 accum_out=csum[:, 1:2])

            # UC = cst2 * (csum0 + csum1); only the stt is on the critical path
            UCa = sp.tile([128, 2], F32, tag="UCa")
            nc.gpsimd.tensor_scalar(UCa[:], cst2[:], csum[:, 0:1],
                                    None, ALU.mult)
            UC = sp.tile([128, 2], BF16, tag="UC")
            nc.vector.scalar_tensor_tensor(UC[:], cst2[:], csum[:, 1:2], UCa[:],
                                           ALU.mult, ALU.add)

            pT = ps_T.tile([128, 16 * BPC], F32, tag="pT")
            for b in range(BPC):
                for j in range(8):
                    lh = CS[64 * b:64 * (b + 1), 128 * j:128 * (j + 1)]
                    nc.tensor.matmul(pT[:, 16 * b + 2 * j: 16 * b + 2 * j + 2],
                                     lh, UC[64 * b:64 * (b + 1), :],
                                     start=True, stop=True)
            e = sp.tile([128, 16 * BPC], F32, tag="e")
            nc.vector.tensor_copy(e[:], pT[:])
            nc.sync.dma_start(out_t[:], e[:])
    return nc


def kernel(x, shift0, shift1, amp0, amp1):
    x = np.asarray(x, dtype=np.float32)
    cst2 = _host_constants(
        np.asarray(shift0).reshape(-1)[0], np.asarray(shift1).reshape(-1)[0],
        np.asarray(amp0).reshape(-1)[0], np.asarray(amp1).reshape(-1)[0])
    nc = _build_program()
    nc.finalize()

    t_full = (x.astype(np.float64) / (2.0 * np.pi)).astype(np.float32)
    in_maps = []
    for c in range(NCORES):
        t_ext = _pack_t(t_full[BPC * c: BPC * (c + 1)])
        in_maps.append({"t": t_ext, "cst2": cst2})
    res = run_bass_kernel_spmd(nc, in_maps, list(range(NCORES)))
    outs = []
    for c in range(NCORES):
        arr = res.results[c]["out"]                      # [128, 16*BPC]
        arr = arr.reshape(128, BPC, 8, 2)                # (p, b, j, i)
        outs.append(arr.transpose(1, 2, 0, 3).reshape(BPC, N, 2))
    return np.concatenate(outs, axis=0).astype(np.float32)


# revision 27
# speedup vs baseline: 1.1235x; 1.0158x over previous
"""Trainium2 Bass kernel for nn_NUFFTLayerMultiChannelInitMixed.

Math: the reference's spread->FFT->filter->IFFT->energy pipeline collapses to
an analytic-spectrum bilinear form. The Gaussian spread is deconvolved exactly
by the deconv^2 filter, so with ghat_n(k) ~ e^{-i k x_n} (alias images carry
weight e^{-tau(M-k)^2} ~ 3e-5 -- negligible vs the 2e-2 gate):

  e_i[n] = sum_k G_i(k) [cos(k x_n) C(k) + sin(k x_n) S(k)] + off_i
  C(k) = sum_n cos(k x_n),  S(k) = sum_n sin(k x_n)
  G_i = pref * w * deconv2 * mult_i * p^2  (~1/k^2 decay; K=32 keeps ~2e-4)

Layout: with K=32, BOTH batches pack into one [128, 1024] trig matrix --
row r: batch r//64, kind (r%64)//32 (cos/sin), k = r%32. One K=7 bf16 phase
matmul per 512 cols builds k*t (3-way bf16 split of t, exact in fp32 PSUM)
with the +1/4-turn cos bias riding in the contraction. Range-reduce
(round-to-nearest via +MAGIC, ACT-led and DVE-led halves to balance engines),
one Sin activation per half (bf16 out, accum_out = row sums for free), then
16 matmuls (stationary = 64-row trig chunk, rhs = G*[C;S] [64, 2]) yield
energies directly in [n-part, channel] layout. The constant per-channel
offset rides the two identically-1 trig rows (cos k=0, and sin k=0 via bias).
Sharding: batch-parallel, 2 of 16 batches per core, no collectives.
"""

import numpy as np

try:
    import concourse.bass as bass
except ImportError:
    import sys
    sys.path.insert(0, "/opt/trn_rl_repo")
    import concourse.bass as bass

import concourse.bacc as bacc
import concourse.mybir as mybir
from concourse import tile
from concourse.bass_utils import run_bass_kernel_spmd

F32 = mybir.dt.float32
BF16 = mybir.dt.bfloat16
AF = mybir.ActivationFunctionType
ALU = mybir.AluOpType

M = 2001
L = 2.0 * np.pi
TAU = 12.0 * (L / (2.0 * np.pi * M)) ** 2
K = 32                   # spectral truncation (1/k^2 filter decay)
N = 1024
B_FULL = 16
NCORES = 8
BPC = B_FULL // NCORES   # batches per core, packed into row halves
MAGIC = 12582912.0       # 1.5 * 2^23: (u + MAGIC) - MAGIC = round-to-nearest(u)
PI = float(np.pi)

_RB = np.arange(128) % 64            # within-batch row index
_KROW = _RB % K                      # k value per row
_BIAS = np.where(_RB <= K, 0.25, 0.0)  # cos rows + the sin k=0 offset row


def _bf16(a):
    a32 = np.asarray(a, dtype=np.float32)
    u32 = a32.view(np.uint32).astype(np.uint64)
    return (((u32 + 0x7FFF + ((u32 >> 16) & 1)) & 0xFFFF0000)
            .astype(np.uint32)).view(np.float32)


def _host_constants(shift0, shift1, amp0, amp1):
    """fp64 host-side k-space weights -> cst2 [128, 2]."""
    k = np.arange(K, dtype=np.float64)
    tau = float(TAU)
    p2 = np.exp(-2.0 * tau * k * k)
    deconv2 = (np.pi / tau) * np.exp(2.0 * tau * k * k)
    mult1 = float(amp0) * (4.0 * np.pi) / (k * k + (1.0 * float(shift0)) ** 2)
    mult2 = float(amp1) * (4.0 * np.pi) / (k * k + (0.5 * float(shift1)) ** 2)
    w = np.full(K, 2.0)
    w[0] = 1.0
    Cc = (M / L) * np.sqrt(4.0 * np.pi * tau)
    scale = 1.0 / ((2.0 * np.pi * M / L) * (2.0 * np.pi))
    pref = scale * Cc * Cc / M
    G1 = pref * w * deconv2 * mult1 * p2
    G2 = pref * w * deconv2 * mult2 * p2

    cst2 = np.zeros((128, 2), dtype=np.float64)
    cst2[:, 0] = G1[_KROW]
    cst2[:, 1] = G2[_KROW]

    # Constant offset off_i = G_i[0]*N - sum(G_i) rides the two rows that are
    # identically 1: cos k=0 (rb=0, bf16-representable part) and sin k=0
    # (rb=K, made 1 by its +0.25 bias; carries the residual) -- no separate
    # offset-add instruction and no bf16 precision loss.
    off1 = float(G1[0] * N - G1.sum())
    off2 = float(G2[0] * N - G2.sum())
    for i, off in enumerate((off1, off2)):
        hi = float(_bf16(np.float32(off)))
        cst2[_RB == 0, i] = hi / N
        cst2[_RB == K, i] = (off - hi) / N
    return cst2.astype(np.float32)


def _pack_t(t_rows):
    """[BPC, N] fp32 t values -> [7, 128+N] bf16: the phase stationary
    [7, 128] at cols 0:128 (first in the DMA), then per-batch 3-way split
    rows of t (rows 3b..3b+2) + ones row 6."""
    ext = np.ones((7, 128 + N), dtype=np.float32)
    for b in range(BPC):
        t = t_rows[b]
        th = _bf16(t)
        tm = _bf16(t.astype(np.float64) - th.astype(np.float64))
        tl = _bf16(t.astype(np.float64) - th.astype(np.float64)
                   - tm.astype(np.float64))
        ext[3 * b + 0, 128:] = th
        ext[3 * b + 1, 128:] = tm
        ext[3 * b + 2, 128:] = tl
    kvb = np.zeros((7, 128), dtype=np.float64)
    for b in range(BPC):
        rows = (np.arange(128) // 64) == b
        for j in range(3):
            kvb[3 * b + j, rows] = _KROW[rows]
    kvb[6] = _BIAS
    ext[:, :128] = kvb
    import ml_dtypes
    return ext.astype(ml_dtypes.bfloat16)


def _build_program(debug=False):
    nc = bacc.Bacc(None, target_bir_lowering=False, debug=debug)
    t_in = nc.declare_dram_parameter("t", [7, 128 + N], BF16, isOutput=False)
    cst_in = nc.declare_dram_parameter("cst2", [128, 2], F32, isOutput=False)
    out_t = nc.declare_dram_parameter("out", [128, 16 * BPC], F32, isOutput=True)

    with tile.TileContext(nc) as tc:
        import contextlib
        with contextlib.ExitStack() as ctx:
            pc = ctx.enter_context(tc.tile_pool(name="const", bufs=1))
            wp = ctx.enter_context(tc.tile_pool(name="work", bufs=2))
            sp = ctx.enter_context(tc.tile_pool(name="small", bufs=1))
            ps_u = ctx.enter_context(tc.tile_pool(name="psu", bufs=2, space="PSUM"))
            ps_T = ctx.enter_context(tc.tile_pool(name="psT", bufs=1, space="PSUM"))

            # Dummy Sin on scratch: makes the FIRST ScalarE op a Sin so the
            # compiler resident-set pick contains sin (its sets also contain
            # identity), avoiding a 1.3us mid-pipeline ACT_TABLE_LOAD swap.
            dummy = sp.tile([1, 2], F32, tag="dummy")
            nc.vector.memset(dummy[:], 0.0)
            dummy2 = sp.tile([1, 2], F32, tag="dummy2")
            nc.scalar.activation(dummy2[:], dummy[:], AF.Sin, scale=1.0)
            magicc = pc.tile([128, 1], F32, tag="magic")
            nc.gpsimd.memset(magicc[:], MAGIC)

            t_ext = pc.tile([7, 128 + N], BF16, tag="t")
            nc.sync.dma_start(t_ext[:, 0:640], t_in[:, 0:640])
            nc.sync.dma_start(t_ext[:, 640:], t_in[:, 640:])
            cst2 = pc.tile([128, 2], F32, tag="cst2")
            nc.sync.dma_start(cst2[:], cst_in[:])
            kvb = t_ext[:, 0:128]

            CS = sp.tile([128, N], BF16, tag="CS")
            csum = sp.tile([128, 2], F32, tag="csum")

            u0 = ps_u.tile([128, 512], F32, tag="u")
            nc.tensor.matmul(u0[:], kvb, t_ext[:, 128:640], start=True, stop=True)
            u1 = ps_u.tile([128, 512], F32, tag="u")
            nc.tensor.matmul(u1[:], kvb, t_ext[:, 640:1152], start=True, stop=True)

            # half 0: DVE-led range reduction (lands first; DVE drains it
            # while ACT handles half 1's Identity in parallel)
            rni = wp.tile([128, 512], F32, tag="rni")
            nc.vector.tensor_scalar(rni[:], u0[:], MAGIC, MAGIC,
                                    ALU.add, ALU.subtract)
            r = wp.tile([128, 512], F32, tag="r")
            nc.vector.tensor_sub(r[:], u0[:], rni[:])
            nc.scalar.activation(CS[:, 0:512], r[:], AF.Sin, scale=2.0 * PI,
                                 accum_out=csum[:, 0:1])
            # half 1: ACT-led (Identity reads PSUM, adds MAGIC)
            rniM = wp.tile([128, 512], F32, tag="rniM")
            nc.scalar.activation(rniM[:], u1[:], AF.Identity, bias=magicc[:])
            negr = wp.tile([128, 512], F32, tag="negr")
            nc.vector.scalar_tensor_tensor(negr[:], rniM[:], MAGIC, u1[:],
                                           ALU.subtract, ALU.subtract)
            nc.scalar.activation(CS[:, 512:1024], negr[:], AF.Sin,
                                 scale=-2.0 * PI, accum_out=csum[:, 1:2])

            # UC = cst2 * (csum0 + csum1); only the stt is on the critical path
            UCa = sp.tile([128, 2], F32, tag="UCa")
            nc.gpsimd.tensor_scalar(UCa[:], cst2[:], csum[:, 0:1],
                                    None, ALU.mult)
            UC = sp.tile([128, 2], BF16, tag="UC")
            nc.vector.scalar_tensor_tensor(UC[:], cst2[:], csum[:, 1:2], UCa[:],
                                           ALU.mult, ALU.add)

            pT = ps_T.tile([128, 16 * BPC], F32, tag="pT")
            e = sp.tile([128, 16 * BPC], F32, tag="e")
            for b in range(BPC):
                for j in range(8):
                    lh = CS[64 * b:64 * (b + 1), 128 * j:128 * (j + 1)]
                    nc.tensor.matmul(pT[:, 16 * b + 2 * j: 16 * b + 2 * j + 2],
                                     lh, UC[64 * b:64 * (b + 1), :],
                                     start=True, stop=True)
                nc.vector.tensor_copy(e[:, 16 * b:16 * (b + 1)],
                                      pT[:, 16 * b:16 * (b + 1)])
                nc.sync.dma_start(out_t[:, 16 * b:16 * (b + 1)],
                                  e[:, 16 * b:16 * (b + 1)])
    return nc


def kernel(x, shift0, shift1, amp0, amp1):
    x = np.asarray(x, dtype=np.float32)
    cst2 = _host_constants(
        np.asarray(shift0).reshape(-1)[0], np.asarray(shift1).reshape(-1)[0],
        np.asarray(amp0).reshape(-1)[0], np.asarray(amp1).reshape(-1)[0])
    nc = _build_program()
    nc.finalize()

    t_full = (x.astype(np.float64) / (2.0 * np.pi)).astype(np.float32)
    in_maps = []
    for c in range(NCORES):
        t_ext = _pack_t(t_full[BPC * c: BPC * (c + 1)])
        in_maps.append({"t": t_ext, "cst2": cst2})
    res = run_bass_kernel_spmd(nc, in_maps, list(range(NCORES)))
    outs = []
    for c in range(NCORES):
        arr = res.results[c]["out"]                      # [128, 16*BPC]
        arr = arr.reshape(128, BPC, 8, 2)                # (p, b, j, i)
        outs.append(arr.transpose(1, 2, 0, 3).reshape(BPC, N, 2))
    return np.concatenate(outs, axis=0).astype(np.float32)
